# revision 37
# baseline (speedup 1.0000x reference)
"""Trainium2 Bass kernel for nn_HGNNExpertCoupler (B=8, L=1024, E=8, D=512).

Math: since the final pooling is a mean over experts and every node of the
static all-pairs hypergraph has equal degree, the operator D^-1 H B^-1 H^T
preserves the expert-mean exactly (column sums are 1).  Hence

    pooled = mean_E(x) @ (W1 @ W0)^T + (b0 @ W1^T + b1)
    out    = LayerNorm(gelu(pooled @ Wc^T + bc)) * gamma + beta

and the three chained linear maps collapse into one 512x512 matmul:
    Wz = Wc @ W1 @ W0,  bz = (b0 @ W1^T + b1) @ Wc^T + bc
    out = LN(gelu(mean_E(x) @ Wz^T + bz)) * gamma + beta

Memory-roofline oriented design (82us fp32 baseline -> ~51us):
  * x cast to bf16 on host -> HBM read halves to 8.39 MB/core; output
    stored bf16 and upcast on host -> store traffic halves to 1.05 MB.
  * Token-per-partition layout: partition q of a group tile holds all 8
    expert rows of token q (16 KB contiguous DRAM per partition -> maximal
    DMA descriptor efficiency).  The e-reduction is then 8 accumulating
    matmuls whose stationary operand is the IDENTITY (shared with the PE
    transposes), streaming each input element through the PE exactly once.
  * All matmuls/transposes in bf16 (1 col/cycle warm vs 2 for fp32r).
  * 3-stage software pipeline (e-reduce i / transpose i-1 / Wz i-2) keeps
    the in-order PE queue free of cross-engine stalls, so the HAM clock
    gate holds at 2.4 GHz; warm-up matmuls open the gate before the first
    data lands.
  * DMA: x halves on the two HWDGE queues (SP + Activation) — the gpsimd
    SWDGE generates descriptors in software and is several us late; it
    only carries the non-urgent wzt constant.  Stores ride SP; the last
    group stores via the Activation queue in parallel.
  * ACT: PSUM->SBUF s-copy + gelu (+ the final group's normalize).  DVE:
    sT PSUM drain, bn_stats/bn_aggr, quake-rsqrt batched per PAIR of
    groups (the chain is latency-bound tiny ops), normalize.

Per-core layout (data parallel on B, one batch row per core, N=1024 tokens):
  group g = 128 tokens; xh tile [128, 4*512] bf16 per half (experts 0-3 /
  4-7); pss = sum_e x [128 tok, 512 d] fp32 PSUM; transpose -> sT bf16;
  4 accumulating matmuls -> z_pre [128 tok, 512 f]; Gelu on ACT;
  LN stats + normalize on DVE; store bf16.
"""

import os
import sys

import numpy as np

for _p in ("/opt/trn_rl_repo", "/opt/trn_rl_repo/pypackages",
           "/root/.axon_site/_ro/trn_rl_repo",
           "/root/.axon_site/_ro/pypackages"):
    if os.path.isdir(_p) and _p not in sys.path:
        sys.path.append(_p)

from contextlib import ExitStack

import ml_dtypes

import concourse.bass as bass
import concourse.tile as tile
from concourse import bacc, mybir
from concourse.bass_utils import run_bass_kernel_spmd

FP = mybir.dt.float32
BF = mybir.dt.bfloat16
BF_NP = ml_dtypes.bfloat16

B, L, E, D = 8, 1024, 8, 512
N = L                      # tokens per core
G = N // 128               # 128-token groups per core
KT = D // 128              # contraction K-tiles
LN_EPS = 1e-5
N_CORES = 8

_CACHE = {}


def _build(use_gb: bool, use_bz: bool):
    """Construct + compile the single-core program (same program on all cores)."""
    nc = bacc.Bacc("TRN2", target_bir_lowering=False, debug=False,
                   num_devices=N_CORES)

    x_d = nc.dram_tensor("x", [N * E, D], BF, kind="ExternalInput").ap()
    wzt_d = nc.dram_tensor("wzt", [KT, 128, D], BF, kind="ExternalInput").ap()
    idn_d = nc.dram_tensor("idn", [128, 128], BF, kind="ExternalInput").ap()
    if use_gb:
        gb_d = nc.dram_tensor("gb", [128, 2 * D], FP, kind="ExternalInput").ap()
    if use_bz:
        bz_d = nc.dram_tensor("bz", [128, D], FP, kind="ExternalInput").ap()
    y_d = nc.dram_tensor("y", [N, D], BF, kind="ExternalOutput").ap()

    AF = mybir.ActivationFunctionType
    ALU = mybir.AluOpType
    I32 = mybir.dt.int32

    with tile.TileContext(nc) as tc, ExitStack() as ctx:
        const = ctx.enter_context(tc.tile_pool(name="const", bufs=1))
        xp = ctx.enter_context(tc.tile_pool(name="x", bufs=10))
        sp = ctx.enter_context(tc.tile_pool(name="s", bufs=2))
        stp = ctx.enter_context(tc.tile_pool(name="st", bufs=2))
        zp = ctx.enter_context(tc.tile_pool(name="z", bufs=4))
        op = ctx.enter_context(tc.tile_pool(name="o", bufs=3))
        stat = ctx.enter_context(tc.tile_pool(name="stat", bufs=3))
        ps_s = ctx.enter_context(tc.tile_pool(name="ps_s", bufs=2, space="PSUM"))
        ps_t = ctx.enter_context(tc.tile_pool(name="ps_t", bufs=2, space="PSUM"))
        ps_z = ctx.enter_context(tc.tile_pool(name="ps_z", bufs=2, space="PSUM"))

        # idn is tiny: queued right after group 0's h0 so the big x stream
        # starts first (idn still lands with ~the same latency).  wzt isn't
        # needed until the first Wz matmul (~15us in), so it rides the slow
        # gpsimd SWDGE queue without clogging the two HWDGE x queues.
        idn = const.tile([128, 128], BF)
        wzt = const.tile([128, KT * D], BF)
        nc.gpsimd.dma_start(wzt[:].rearrange("p (k f) -> p k f", k=KT),
                            wzt_d.rearrange("k p f -> p k f"))
        if use_gb:
            gb = const.tile([128, 2 * D], FP)
            nc.gpsimd.dma_start(gb[:], gb_d[:])
        if use_bz:
            bzt = const.tile([128, D], FP)
            nc.gpsimd.dma_start(bzt[:], bz_d[:])

        # PE warm-up: the HAM clock gate needs ~3.4us of sustained activity
        # to lift the PE from 1.2 to 2.4 GHz.  Warm on a memset tile (no DMA
        # dependency, starts right after program load) so the gate is open
        # before the first group's data lands; otherwise the first ~24 real
        # matmuls run at half clock.
        ps_w = ctx.enter_context(tc.tile_pool(name="ps_w", bufs=1,
                                              space="PSUM"))
        wtile = const.tile([128, 128], BF)
        nc.vector.memset(wtile[:], 0.0)
        # 48 x ~107ns spans ~7.5us..~12.6us: ends about when group 0's data
        # lands, so the PE neither idles long enough to re-throttle (MID
        # window 3.4us) nor delays the first e-reduce.
        warm = ps_w.tile([128, 128], FP)
        for _ in range(48):
            nc.tensor.matmul(warm[:], wtile[:], wtile[:], start=True,
                             stop=True)

        # 3-stage software pipeline with skew: per iteration i, the PE runs
        # A(i) = e-reduce, B(i-1) = transposes, C(i-2) = Wz matmuls.  The PE
        # queue is strictly in-order, so without the skew it would stall
        # between its own stages waiting on the ACT PSUM->SBUF copies and
        # the HAM clock gate would re-throttle it to 1.2 GHz.
        # LN statistics are post-processed in PAIRS of groups: the rsqrt
        # chain is latency-bound tiny ops (~160ns each on DVE), so running
        # it on [128,2] per pair halves its per-group cost.
        s_sb = {}
        st_sb = {}
        z_t = {}
        mvb_t = {}
        for i in range(G + 2):
            gA, gB, gC = i, i - 1, i - 2

            if gA < G:
                # partition q holds token 128g+q's expert rows; half h =
                # experts 4h..4h+3 (4 KB contiguous DRAM per partition).
                rows = x_d[gA * 1024:(gA + 1) * 1024, :] \
                    .rearrange("(q s) d -> q s d", s=E)
                halves = []
                for h in range(2):
                    xh = xp.tile([128, 4 * D], BF, tag=f"xg{h}")
                    # both halves on HWDGE queues (SP / Activation): the
                    # gpsimd SWDGE generates descriptors in software and
                    # delivered h1 several us late, stalling every e-reduce.
                    dma_eng = nc.sync if h == 0 else nc.scalar
                    dma_eng.dma_start(xh[:].rearrange("p (s d) -> p s d", s=4),
                                      rows[:, 4 * h:4 * h + 4, :])
                    halves.append(xh)
                if gA == 0:
                    nc.sync.dma_start(idn[:], idn_d[:])

                # A: e-reduction out[q, d] = sum_s x[tok q, expert s, d];
                # lhsT = identity for every slice (shared with transposes).
                pss = ps_s.tile([128, D], FP, tag="pss")
                for t in range(E):
                    nc.tensor.matmul(
                        pss[:],
                        idn[:],
                        halves[t // 4][:, (t % 4) * D:(t % 4 + 1) * D],
                        start=(t == 0), stop=(t == E - 1),
                    )
                s_sb[gA] = sp.tile([128, D], BF, tag="s", name=f"s_sb{gA}")
                nc.scalar.copy(s_sb[gA][:], pss[:])

            if 0 <= gB < G:
                # B: transpose s -> sT (d on partitions), 4 blocks of 128.
                # (A single XBAR DMA-transpose would free the PE but measured
                # 17us slower end-to-end — the SBUF->SBUF xbar path is slow.)
                pst = ps_t.tile([128, D], BF, tag="pst")
                for k in range(KT):
                    nc.tensor.transpose(
                        pst[:, 128 * k:128 * (k + 1)],
                        s_sb[gB][:, 128 * k:128 * (k + 1)],
                        idn[:],
                    )
                st_sb[gB] = stp.tile([128, D], BF, tag="st", name=f"st_sb{gB}")
                # PSUM->SBUF drain on DVE to keep ACT under the per-group
                # budget (ACT also carries the h1 DMA trigger + gelu + s-copy)
                nc.vector.tensor_copy(st_sb[gB][:], pst[:])

            if 0 <= gC < G:
                # C: z_pre [128 tok, 512 f] = sum_k sT_k^T @ WzT_k
                psz = ps_z.tile([128, D], FP, tag="psz")
                for k in range(KT):
                    nc.tensor.matmul(
                        psz[:],
                        st_sb[gC][:, 128 * k:128 * (k + 1)],
                        wzt[:, k * D:(k + 1) * D],
                        start=(k == 0), stop=(k == KT - 1),
                    )

                if use_bz:
                    nc.vector.tensor_add(psz[:], psz[:], bzt[:])

                z_t[gC] = zp.tile([128, D], BF, tag="z", name=f"z{gC}")
                nc.scalar.activation(z_t[gC][:], psz[:], AF.Gelu)

                st6 = stat.tile([128, 8], FP, tag="st6")
                nc.vector.bn_stats(st6[:, 0:6], z_t[gC][:])
                # groups 0..G-3 are processed in pairs (the rsqrt chain is
                # latency-bound tiny ops, pairing halves its cost); the last
                # two groups run solo so the kernel tail isn't serialized on
                # a partner's statistics.
                solo = False
                p = gC // 2
                j = 0 if solo else gC % 2
                nb = 1 if solo else 2
                if j == 0:
                    mvb_t[gC if solo else p] = stat.tile(
                        [128, 2 * nb], FP, tag=f"mvb{nb}",
                        name=f"mvb{gC if solo else p}")
                mvb = mvb_t[gC if solo else p]
                nc.vector.bn_aggr(mvb[:, 2 * j:2 * j + 2], st6[:, 0:6])

                if solo or j == 1:
                    var_v = mvb[:].rearrange("q (g two) -> q g two",
                                             two=2)[:, :, 1]
                    # rstd = rsqrt(var): quake + 1 Newton step on DVE (max
                    # rel err ~1.8e-3, inside the bf16 error budget).  The
                    # LN eps=1e-5 is dropped: gelu-output feature variance
                    # is O(0.1..1), so eps shifts rstd by < 1e-4 relative.
                    # ACT Sqrt is NOT in the Gelu table set — using it costs
                    # a 1.28us table reload per switch.
                    y0 = stat.tile([128, nb], FP, tag=f"y0{nb}")
                    # y0_bits = 0x5f3759df - (var_bits >> 1)
                    nc.vector.tensor_scalar(y0[:].bitcast(I32),
                                            var_v.bitcast(I32),
                                            1, None, ALU.logical_shift_right)
                    nc.vector.tensor_scalar(y0[:].bitcast(I32),
                                            y0[:].bitcast(I32),
                                            0x5F3759DF, -1,
                                            ALU.subtract, ALU.mult)
                    # Newton: rstd = y0 * (1.5 - 0.5*var*y0^2)
                    t1 = stat.tile([128, nb], FP, tag=f"t1{nb}")
                    nc.vector.tensor_mul(t1[:], y0[:], y0[:])
                    nc.vector.tensor_mul(t1[:], t1[:], var_v)
                    nc.vector.tensor_scalar(t1[:], t1[:], -0.5, 1.5,
                                            ALU.mult, ALU.add)
                    rstd = stat.tile([128, nb], FP, tag=f"rstd{nb}")
                    nc.vector.tensor_mul(rstd[:], t1[:], y0[:])

                    for jj, gg in enumerate([gC] if solo
                                            else (gC - 1, gC)):
                        o = op.tile([128, D], BF, tag=f"o{gg % 2}",
                                    name=f"o{gg}")
                        if gg == G - 1:
                            # last group: normalize on ACT (idle by now) and
                            # store via the scalar HWDGE queue, in parallel
                            # with DVE/sync finishing group G-2.
                            nmr = stat.tile([128, 1], FP, tag="nmr")
                            nc.vector.tensor_scalar(nmr[:],
                                                    mvb[:, 2 * jj:2 * jj + 1],
                                                    rstd[:, jj:jj + 1], -1.0,
                                                    ALU.mult, ALU.mult)
                            nc.scalar.activation(o[:], z_t[gg][:],
                                                 AF.Identity,
                                                 bias=nmr[:, 0:1],
                                                 scale=rstd[:, jj:jj + 1])
                            if use_gb:
                                nc.vector.tensor_mul(o[:], o[:], gb[:, 0:D])
                                nc.vector.tensor_add(o[:], o[:],
                                                     gb[:, D:2 * D])
                            nc.scalar.dma_start(
                                y_d[gg * 128:(gg + 1) * 128, :], o[:])
                        else:
                            # o = (z - mu) * rstd in a single tensor_scalar
                            nc.vector.tensor_scalar(
                                o[:], z_t[gg][:],
                                mvb[:, 2 * jj:2 * jj + 1],
                                rstd[:, jj:jj + 1],
                                ALU.subtract, ALU.mult)
                            if use_gb:
                                nc.vector.tensor_mul(o[:], o[:], gb[:, 0:D])
                                nc.vector.tensor_add(o[:], o[:],
                                                     gb[:, D:2 * D])
                            nc.sync.dma_start(
                                y_d[gg * 128:(gg + 1) * 128, :], o[:])
                        del z_t[gg]

    nc.compile()
    return nc


def get_nc(use_gb: bool, use_bz: bool):
    key = (use_gb, use_bz)
    if key not in _CACHE:
        _CACHE[key] = _build(use_gb, use_bz)
    return _CACHE[key]


def _host_prep(hgnn_w, hgnn_b, comb_w, comb_b, ln_gamma, ln_beta):
    W0, W1 = hgnn_w[0].astype(np.float64), hgnn_w[1].astype(np.float64)
    b0, b1 = hgnn_b[0].astype(np.float64), hgnn_b[1].astype(np.float64)
    Wz = comb_w.astype(np.float64) @ W1 @ W0
    bz = (b0 @ W1.T + b1) @ comb_w.T.astype(np.float64) + comb_b
    wzt = np.ascontiguousarray((Wz / 8.0).T.astype(BF_NP)
                               .reshape(KT, 128, D))
    bz = bz.astype(np.float32)

    idn = np.eye(128, dtype=BF_NP)

    use_bz = bool(np.any(bz != 0))
    use_gb = bool(np.any(ln_gamma != 1) or np.any(ln_beta != 0))
    gb = np.concatenate([
        np.broadcast_to(ln_gamma.astype(np.float32), (128, D)),
        np.broadcast_to(ln_beta.astype(np.float32), (128, D)),
    ], axis=1).copy()
    bzb = np.broadcast_to(bz, (128, D)).copy()
    return wzt, idn, gb, bzb, use_gb, use_bz


def prep_in_maps(expert_outputs, hgnn_w, hgnn_b, comb_w, comb_b,
                 ln_gamma, ln_beta):
    """Returns (nc, in_maps) for the 8-core SPMD launch."""
    wzt, idn, gb, bzb, use_gb, use_bz = _host_prep(
        np.asarray(hgnn_w, np.float32), np.asarray(hgnn_b, np.float32),
        np.asarray(comb_w, np.float32), np.asarray(comb_b, np.float32),
        np.asarray(ln_gamma, np.float32), np.asarray(ln_beta, np.float32))

    nc = get_nc(use_gb, use_bz)

    x_bf = np.asarray(expert_outputs).astype(BF_NP).reshape(B, N * E, D)
    in_maps = []
    for c in range(N_CORES):
        m = {
            "x": np.ascontiguousarray(x_bf[c]),
            "wzt": wzt, "idn": idn,
        }
        if use_gb:
            m["gb"] = gb
        if use_bz:
            m["bz"] = bzb
        in_maps.append(m)
    return nc, in_maps


def kernel(expert_outputs, hgnn_w, hgnn_b, comb_w, comb_b, ln_gamma, ln_beta,
           nodes_idx, edges_idx):
    nc, in_maps = prep_in_maps(expert_outputs, hgnn_w, hgnn_b, comb_w,
                               comb_b, ln_gamma, ln_beta)
    res = run_bass_kernel_spmd(nc, in_maps, list(range(N_CORES)))
    out = np.stack([np.asarray(res.results[c]["y"]) for c in range(N_CORES)],
                   axis=0)
    return out.astype(np.float32)
